# revision 12
# baseline (speedup 1.0000x reference)
"""BioSelfAttention on 8 TRN2 NeuronCores.

Full inputs Q,K,V: (B=2, H=8, T=256, D=64) f32. Data-parallel over the 16
(b,h) pairs: core c owns flat pairs {2c, 2c+1}.

The module constant-folds: its output is the constant 1/16384 for every
finite input, so the device kernel reduces to storing that constant.

Why (exact, not approximate):
  * The WTA update is r <- softmax((r + r @ W.T)/temp) with W = inh*ones
    + (exc-inh)*I, i.e. softmax(3r - 0.9*sum(r)); the -0.9*sum(r) term is
    constant per row and cancels inside softmax, leaving r <- softmax(3r).
    Over N elements this map contracts deviations from uniform by ~3/N per
    step (Jacobian 3(diag(p) - p p^T); globally every state collapses
    toward uniform since exp(3)/(exp(3)+N-1) << 1 for N >= 256). With the
    reference's 20 iterations the state reaches the uniform vector
    BITWISE: once fp32 rounding ties all entries, softmax gives exp(0)=1
    per element and the sum (a power of two: 256 or 16384) is exact, so
    uniform is an exact fixed point of the fp32 computation.
  * WTA1 (N=256) therefore yields rates_inh == 1/256 exactly. Then
    J_v = V/256 with |V| ~ randn, far below the LIF threshold (a spike
    needs J >= 1/(1-0.95^100) ~ 1.006), so context == 0 identically.
  * WTA2 (N=16384) of the all-zero vector: softmax(0) = 1/16384 exactly
    (the sum of 16384 ones is exact), a fixed point of every later
    iteration.
  * Even for huge inputs where J_v does spike, WTA2's 20 iterations
    contract by 3/16384 per step -> bitwise uniform 1/16384 regardless.
    The output is input-independent for all finite inputs.
  Verified bitwise against the jax reference for seeds {0,1,42,123} x
  input scales {1,10,100,300}, and on-device against a full LIF/WTA
  Bass implementation of the pipeline (53 us; rel err 0).

Device kernel (per core): the 128 KiB constant output block is embedded
in the NEFF (Const tensors are DMA'd to HBM at model-load time) and the
kernel issues two contiguous 64 KiB DRAM->DRAM copies into the core's
(2,256,64) output, one on each hardware-DGE engine (SP + Activation) so
descriptor generation overlaps. The .then_inc supplies the DGE sync info
codegen requires; nothing needs to wait on it — the transfers complete
microseconds before the post-halt teardown that gates host readback.

After compile we prune the framework's post-preamble all-engine
gather/release barrier from the emitted main block: the DMAs depend only
on load-time state, and the barrier's two cross-engine semaphore hops
(~1 us) are pure launch latency. If the emitted structure ever changes,
nothing is pruned and the kernel just runs marginally slower.

Measured on 8xTRN2 (trace on core 0): 8.5-8.7 us vs 53.3 us for the
full on-device LIF/WTA pipeline; rel err 0 (bitwise).
"""

import numpy as np
import concourse.bacc as bacc
import concourse.mybir as mybir
from concourse.bass_utils import run_bass_kernel_spmd

F32 = mybir.dt.float32
B, H, T, D = 2, 8, 256, 64
N_CORES = 8
UNIFORM = 1.0 / 16384.0

_NC_CACHE = {}


def _is_barrier(inst):
    si = getattr(inst, "sync_info", None)
    if si is not None:
        for x in list(si.on_wait) + list(si.on_update):
            if "barrier_" in (x.ant_name or ""):
                return True
    return False


def _build_nc():
    if "nc" in _NC_CACHE:
        return _NC_CACHE["nc"]
    nc = bacc.Bacc(None, target_bir_lowering=False, debug=False)
    out = nc.dram_tensor("out", [2, T, D], F32, kind="ExternalOutput")
    csrc = nc.inline_tensor(
        np.full((2, T, D), UNIFORM, dtype=np.float32), name="csrc")
    s0 = nc.alloc_semaphore(name="dma_done0")
    s1 = nc.alloc_semaphore(name="dma_done1")
    nc.sync.dma_start(out=out.ap()[0], in_=csrc.ap()[0]).then_inc(s0, 16)
    nc.scalar.dma_start(out=out.ap()[1], in_=csrc.ap()[1]).then_inc(s1, 16)
    nc.compile()

    # Prune the entry gather/release barrier (and the bare Pool drain
    # feeding it) from our own emitted main block; see module docstring.
    blk = nc.main_func.blocks[0]
    insts = list(blk.instructions)
    keep = []
    for idx, inst in enumerate(insts):
        bare_pool_drain = (
            type(inst).__name__ == "InstDrain"
            and str(getattr(inst, "engine", "")) == "EngineType.Pool"
            and not _is_barrier(inst)
            and idx + 1 < len(insts)
            and _is_barrier(insts[idx + 1]))
        if _is_barrier(inst) or bare_pool_drain:
            continue
        keep.append(inst)
    blk.instructions[:] = keep

    _NC_CACHE["nc"] = nc
    return nc


def _run(Q, K, V, trace=False, **trace_kwargs):
    nc = _build_nc()
    in_maps = [{} for _ in range(N_CORES)]
    res = run_bass_kernel_spmd(nc, in_maps, list(range(N_CORES)),
                               trace=trace, **trace_kwargs)
    out = np.concatenate([res.results[c]["out"] for c in range(N_CORES)],
                         axis=0)
    return out.reshape(B, H, T, D), res


def kernel(Q, K, V):
    out, _ = _run(Q, K, V)
    return out
